# revision 36
# baseline (speedup 1.0000x reference)
"""Multi-head attention forward on 8 TRN2 NeuronCores.

Problem: x[2, 2048, 1024], 16 heads x 64 dims, nn.Linear-style Q/K/V/O
projections.

Sharding: core c owns batch b = c // 4 and heads [4*(c%4), 4*(c%4)+4).
Each core computes Q/K/V projections for its 4 heads over its batch's
2048 tokens, attention, and a partial O-projection restricted to its
heads' input dims.  The host sums the 4 partials per batch and adds bo.

Precision scheme (validated on HW, end-to-end rel err ~9e-3 vs 2e-2):
  - x ships as e4m3 hi/lo pairs scaled 16x; Q/K/V weights as e4m3
    scaled 64x (the uniform(+-1/32) weights would otherwise sit below
    e4m3's 2^-6 min-normal).  Q/K projections run 2-term compensated
    fp8 DoubleRow matmuls (w_hi*(x_hi + x_lo)); V runs 3-term
    (+ w_lo*x_hi) since it is not re-quantized.  All cheaper than bf16
    (0.5 cycles/row in DoubleRow mode).
  - Q and K are re-quantized to e4m3 at 16x scale on copyback
    (psum * 1/64, Q with fused per-partition bias add); the 2048-wide
    scores matmuls run fp8 DoubleRow with the head dim split as one
    real 64-row plane plus a zeroed "ghost" plane, halving PE time vs
    bf16.  The exp scale absorbs the 256x.
  - K projection bias is dropped entirely: softmax over keys is
    invariant to the per-query q.bk term, and the bq.k term is kept via
    Q's bias.  attn weights (exp) and V stay bf16 -- fp8 there fails
    the error budget.
  - O projection bf16; per-core partials stored bf16, summed f32 host.

On-chip layout / schedule (per core):
  - Q8/K8 "transposed": [hd on partitions, tokens free] per e-tile;
    K8 carries a zeroed 128-col block at the end (stationary ghost
    plane), Q8 a zeroed 256-col pad (moving-operand lookahead).
  - V produced directly as [token, dim] tiles (stationary x trick), no
    PE transposes; a ones-column per head makes the PV matmul emit the
    softmax denominators as row 64 of the ctx PSUM tile.
  - 8 single-head passes (16 m-iterations each); PSUM: two score banks
    (s0/s1), one ctx accumulator (cx), two 1-bank projection slots
    (p0/p1).  Projection/O-tile "fillers" are paced into the PE queue by
    a budget, with issue-order dependencies enforced by require().
    Each pass defers its last PVs and its normalization into the next
    pass so no pass ends in a drain burst that starves the ACT engine
    (exp is the second-busiest engine and must run back-to-back).
  - denominator reciprocals (bf16) are broadcast across 64 partitions
    with a K=1 ones matmul, ctx normalized into CT, then O-projection.
"""

import json
import os
from contextlib import ExitStack

import ml_dtypes
import numpy as np

import concourse.bass as bass
import concourse.tile as tile
from concourse import mybir

BF16 = mybir.dt.bfloat16
F8 = mybir.dt.float8e4
F32 = mybir.dt.float32
AF = mybir.ActivationFunctionType
DR = mybir.MatmulPerfMode.DoubleRow
NPBF16 = ml_dtypes.bfloat16
NP8 = ml_dtypes.float8_e4m3

P = 128
B = 2
NTOK = 2048          # tokens per core (one batch)
ED = 1024
KD = ED // P         # 8 contraction k-tiles for projections
NE = 2               # e-tiles per core (4 heads * 64 = 256 dims)
NH_CORE = 4          # heads per core
HD = 64
MT = NTOK // P       # 16 key/value m-tiles
NCHUNK = 1024        # query-token chunk for the attention inner loop
VROW = NH_CORE * 65  # V tile row: 4x (64 dims + ones column)
XS, WS = 16.0, 64.0  # fp8 pre-scales on x and w
PSC = XS * WS        # projection psum scale
EXPSC = 0.125 / (XS * XS)  # exp scale: 1/sqrt(64) / (16*16)


def _mha_body(ctx: ExitStack, tc: tile.TileContext, outs: dict, ins: dict):
    nc = tc.nc
    xh, xl = ins["xh"], ins["xl"]                   # [128, 8, 2048] fp8
    wq, wk = ins["wqh"], ins["wkh"]                 # [128, 8, 256] fp8
    wv = (ins["wvh"], ins["wvl"])
    wo = ins["wo"]          # [128, 2, 1024] bf16   [p, k, d] = wo[d, e0+128k+p]
    bq, bv = ins["bq"], ins["bv"]                   # [1, 256] bf16, x1024
    out = outs["out"]       # [2048, 1024] f32

    const = ctx.enter_context(tc.tile_pool(name="const", bufs=1))
    sb_big = ctx.enter_context(tc.tile_pool(name="sb_big", bufs=1))
    sb_ex = ctx.enter_context(tc.tile_pool(name="sb_ex", bufs=14))
    sb_sm = ctx.enter_context(tc.tile_pool(name="sb_sm", bufs=4))
    sb_out = ctx.enter_context(tc.tile_pool(name="sb_out", bufs=4))
    psum = ctx.enter_context(tc.tile_pool(name="psum", bufs=1, space="PSUM"))

    # ---- SBUF homes ----
    xh_sb = sb_big.tile([P, KD, NTOK], F8)
    xl_sb = sb_big.tile([P, KD, NTOK], F8)
    wqh_sb = const.tile([P, KD, 2 * P], F8)
    wkh_sb = const.tile([P, KD, 2 * P], F8)
    wvh_sb = const.tile([P, KD, 2 * P], F8)
    wvl_sb = const.tile([P, KD, 2 * P], F8)
    wo_sb = const.tile([P, NE, ED], BF16)
    bqc_sb = const.tile([P, NE], F32)
    bv_sb = const.tile([1, 2 * P], BF16)

    ones_row = const.tile([1, 512], BF16)
    Q8 = sb_big.tile([P, NE, NTOK + 256], F8)   # pad: moving ghost lookahead
    K8 = sb_big.tile([P, NE, NTOK + P], F8)     # pad: stationary ghost zeros
    V = sb_big.tile([P, MT, VROW], BF16)
    CT = sb_big.tile([P, NE, NTOK], BF16)  # normalized ctxT[e, n]
    nc.vector.memset(ones_row, 1.0)
    nc.vector.memset(V[:, :, 64::65], 1.0)
    nc.vector.memset(Q8[:, :, NTOK:], 0.0)
    nc.vector.memset(K8[:, :, NTOK:], 0.0)

    # ---- input DMA on two parallel HWDGE rings (SP + ACT), batched and
    # ordered so the critical prologue path (wk, wq, x tokens 0:1024) lands
    # first on both rings.
    nc.sync.dma_start(xh_sb[:, :, 0:512], xh[:, :, 0:512])
    nc.sync.dma_start(xl_sb[:, :, 0:512], xl[:, :, 0:512])
    nc.sync.dma_start(bqc_sb, bq)
    nc.sync.dma_start(bv_sb, bv)
    nc.sync.dma_start(wvh_sb, wv[0])
    nc.sync.dma_start(wvl_sb, wv[1])
    nc.sync.dma_start(xh_sb[:, :, 1024:1536], xh[:, :, 1024:1536])
    nc.sync.dma_start(xl_sb[:, :, 1024:1536], xl[:, :, 1024:1536])
    nc.scalar.dma_start(wqh_sb, wq)
    nc.scalar.dma_start(wkh_sb, wk)
    nc.scalar.dma_start(xh_sb[:, :, 512:1024], xh[:, :, 512:1024])
    nc.scalar.dma_start(xl_sb[:, :, 512:1024], xl[:, :, 512:1024])
    nc.scalar.dma_start(wo_sb, wo)
    nc.scalar.dma_start(xh_sb[:, :, 1536:2048], xh[:, :, 1536:2048])
    nc.scalar.dma_start(xl_sb[:, :, 1536:2048], xl[:, :, 1536:2048])


    pj = [0]
    TERMS = ((0, 0), (0, 1), (1, 0))  # (w, x) hi/lo compensation pairs
    cfg = json.loads(os.environ.get("KTUNE", "null")) or {
        "bud": [[1.8, 0.9], [1.0, 0.9], [0.9, 0.8], [0.9, 0.8],
                [0.9, 0.8], [0.8, 0.8], [1.0, 1.1], [1.0, 1.1]],
        "carry1": 6, "lag2": 7, "lag8": 6, "wrm": 8}

    def q_ghost(t, i, woff):
        """Q8 window as [64, 2, 256] with plane 1 -> the zero pad."""
        s = Q8[64 * i:64 * i + 64, t, woff:woff + 512]
        return bass.AP(tensor=s.tensor, offset=s.offset,
                       ap=[s.ap[0], [NTOK - woff, 2], [1, 256]])

    def k_ghost(t, i, m):
        """K8 m-tile as [64, 2, 128] with plane 1 -> the zero block."""
        s = K8[64 * i:64 * i + 64, t, m * P:(m + 1) * P]
        return bass.AP(tensor=s.tensor, offset=s.offset,
                       ap=[s.ap[0], [NTOK - m * P, 2], [1, P]])

    def proj_qk(w_sb, b_sb, dst, t, n):
        """Q/K projection tile: dst[:, t, n*512:(n+1)*512] fp8 = psum/64.

        2-term compensation: w_hi @ (x_hi + x_lo); the dropped w_lo term
        is below the e4m3 re-quantization noise of the QK matmul."""
        ps = psum.tile([P, 512], F32, tag=f"p{pj[0] % 2}", name="ps_proj")
        pj[0] += 1
        x2 = (xh_sb, xl_sb)
        for b in range(2):
            for w in range(2):
                base = n * 512 + w * 256
                for j in range(KD // 2):
                    nc.tensor.matmul(
                        ps[:, w * 256:(w + 1) * 256],
                        w_sb[:, 2 * j:2 * j + 2, t * P:(t + 1) * P],
                        x2[b][:, 2 * j:2 * j + 2, base:base + 256],
                        start=(b == 0 and w == 0 and j == 0),
                        stop=(b == 1 and w == 1 and j == KD // 2 - 1),
                        perf_mode=DR, skip_group_check=True)
        if b_sb is not None:
            # fused (psum + 1024*bq) / 64 on the copyback
            nc.vector.tensor_scalar(
                dst[:, t, n * 512:(n + 1) * 512], ps,
                b_sb[:, t:t + 1], 1.0 / 64.0,
                mybir.AluOpType.add, mybir.AluOpType.mult)
        else:
            nc.vector.tensor_scalar_mul(
                dst[:, t, n * 512:(n + 1) * 512], ps, 1.0 / 64.0)

    def proj_v(m):
        """V m-tile direct as [token, dim]: V[:, m, :] bf16 = psum/1024."""
        pv = psum.tile([P, 2 * P], F32, tag=f"p{pj[0] % 2}", name="ps_v")
        pj[0] += 1
        x2 = (xh_sb, xl_sb)
        for ti, (a, b) in enumerate(TERMS):
            for j in range(KD // 2):
                nc.tensor.matmul(
                    pv,
                    x2[b][:, 2 * j:2 * j + 2, m * P:(m + 1) * P],
                    (wvh_sb, wvl_sb)[a][:, 2 * j:2 * j + 2, :],
                    start=(ti == 0 and j == 0), stop=False, perf_mode=DR)
        nc.tensor.matmul(pv, ones_row[:, 0:P], bv_sb,
                         start=False, stop=True)
        s = V[:, m, 0:VROW]
        dst = bass.AP(tensor=s.tensor, offset=s.offset,
                      ap=[s.ap[0], [65, NH_CORE], [1, HD]])
        nc.vector.tensor_scalar_mul(dst, pv, 1.0 / PSC)

    def o_tile(c, t, tail):
        r = c * NCHUNK + t * P
        ob = sb_out.tile([P, ED], BF16)
        slots = ("p0", "p1", "s0", "s1") if tail else ("p0", "p1")
        for u in range(2):
            po = psum.tile([P, 512], F32, tag=slots[pj[0] % len(slots)],
                           name="ps_o2")
            pj[0] += 1
            for k in range(NE):
                nc.tensor.matmul(
                    po,
                    CT[:, k, r:r + P],
                    wo_sb[:, k, u * 512:(u + 1) * 512],
                    start=(k == 0), stop=(k == NE - 1))
            if u == 1 and tail:
                # ACT is idle once the last exp has issued
                nc.scalar.copy(ob[:, u * 512:(u + 1) * 512], po)
            else:
                nc.vector.tensor_copy(ob[:, u * 512:(u + 1) * 512], po)
        eng = nc.sync if t % 2 == 0 else nc.scalar
        eng.dma_start(out[r:r + P, :], ob)

    KG = (wkh_sb, None, K8)
    QG = (wqh_sb, bqc_sb, Q8)

    def attn_head(c, et, i, fillers, budget_early, budget, lag=6,
                  prev=None, last=False, carry=4):
        """One head-chunk pass: head 2*et+i over query chunk c.

        Returns (norm, leftover): `leftover` is this pass's last CARRY pv
        closures, `norm` issues its normalization.  The NEXT pass runs the
        leftover pvs in its first two iterations and the norm right after,
        so no pass ends with a PE drain burst that starves ACT, and the
        single cx PSUM buffer is handed over exactly between the norm
        reads and the next first PV (which waits at least `lag` iters)."""
        g = c * NCHUNK
        h = 2 * et + i
        cx = psum.tile([65, NCHUNK], F32, tag="cx", bufs=1, name="ps_cx")
        exs = {}
        pvd = [0]
        own_last = MT if last else MT - carry

        def pv(m):
            ex = exs.pop(m)
            for u in range(2):
                nc.tensor.matmul(
                    cx[:, u * 512:(u + 1) * 512],
                    V[:, m, h * 65:h * 65 + 65],
                    ex[:, u * 512:(u + 1) * 512],
                    start=(m == 0), stop=(m == MT - 1))

        debt = [0.0]
        require(("Q", et, 2 * c))
        require(("Q", et, 2 * c + 1))
        for m in range(MT):
            require(("K", et, m // 4))
            sc = psum.tile([P, NCHUNK], F32, tag=f"s{m % 2}", name="ps_sc")
            for j in range(4):
                nc.tensor.matmul(
                    sc[:, j * 256:(j + 1) * 256],
                    k_ghost(et, i, m),
                    q_ghost(et, i, g + 256 * j),
                    start=(j % 2 == 0), stop=(j % 2 == 1), perf_mode=DR,
                    skip_group_check=True)
            ex = sb_ex.tile([P, NCHUNK], BF16, tag="ex", name="ex")
            nc.scalar.activation(ex, sc, AF.Exp, scale=EXPSC)
            exs[m] = ex
            if prev is not None:
                if prev[1]:
                    for _ in range(2):
                        if prev[1]:
                            prev[1].pop(0)()
                elif prev[0] is not None:
                    prev[0]()
                    prev = (None, ())
            # own PVs: lag behind the exp stream, catching up to two per
            # iteration late in the pass, leaving the last CARRY deferred
            target = min(own_last, m + 1,
                         max(0, m - lag + 1,
                             2 * m - 2 * (MT - 2) + own_last))
            while pvd[0] < target:
                require(("V", pvd[0]))
                pv(pvd[0])
                pvd[0] += 1
            debt[0] += budget_early if m < lag else budget
            # keep the DVE queue clear near the pass end so the
            # normalization chain is not stuck behind filler copybacks
            if (fillers and m < MT - 3 and debt[0] >= fillers[0][0]
                    and (fillers[0][2] is None or fillers[0][2]())):
                debt[0] -= run_filler()
        while last and pvd[0] < MT:
            require(("V", pvd[0]))
            pv(pvd[0])
            pvd[0] += 1
        leftover = []
        for m in range(pvd[0], MT):
            def run_pv(m=m):
                require(("V", m))
                pv(m)
            leftover.append(run_pv)

        def norm_half(v):
            # normalization, pipelined in column halves; the reciprocal row
            # is broadcast across 64 partitions with a K=1 ones matmul (PE)
            pp = i * HD
            vs = slice(v * 512, (v + 1) * 512)
            rcp = sb_sm.tile([1, 512], BF16, tag="rcp", name="rcp")
            with nc.allow_low_precision(reason="bf16 denominators: "
                                        "0.4% rms, inside the budget"):
                nc.vector.reciprocal(rcp, cx[64:65, vs])
            bcp = psum.tile([HD, 512], F32, tag=f"p{pj[0] % 2}",
                            name="ps_bc")
            pj[0] += 1
            nc.tensor.matmul(bcp, ones_row[:, 0:HD], rcp,
                             start=True, stop=True)
            bc = sb_sm.tile([HD, 512], BF16, name="bc")
            nc.vector.tensor_copy(bc, bcp)
            nc.vector.tensor_mul(
                CT[pp:pp + HD, et, g + v * 512:g + v * 512 + 512],
                cx[0:HD, vs], bc)

        def norm():
            norm_half(0)
            norm_half(1)
            normed[0] += 1
        norm.half = norm_half
        return norm, leftover

    # PE warmup: dependency-free dummy matmuls burn through the p-state
    # ramp while the first input DMAs land, so the prologue projections
    # run at full clock
    wrm = psum.tile([P, 512], F32, tag="p0", name="wrm")
    for _ in range(cfg.get("wrm", 8)):
        nc.tensor.matmul(wrm, ones_row[:, 0:P], ones_row,
                         start=True, stop=True)
    pj[0] += 1  # leave p1 for the first projection

    # prologue: the minimum the first pass (head 0, e-tile 0, chunk 0)
    # needs: K(0,0), Q(0,0), Q(0,1).  Emitted with all x_hi phases first so
    # PE is never parked on the later x_lo DMAs.
    x2 = (xh_sb, xl_sb)
    pro = []
    for tag, w_sb, n in (("p0", wqh_sb, 0), ("p1", wkh_sb, 0),
                         ("s0", wqh_sb, 1)):
        pro.append(psum.tile([P, 512], F32, tag=tag, name="ps_proj"))
    for b in range(2):
        for (ps, w_sb, n) in zip(pro, (wqh_sb, wkh_sb, wqh_sb), (0, 0, 1)):
            for w in range(2):
                base = n * 512 + w * 256
                for j in range(KD // 2):
                    nc.tensor.matmul(
                        ps[:, w * 256:(w + 1) * 256],
                        w_sb[:, 2 * j:2 * j + 2, 0:P],
                        x2[b][:, 2 * j:2 * j + 2, base:base + 256],
                        start=(b == 0 and w == 0 and j == 0),
                        stop=(b == 1 and w == 1 and j == KD // 2 - 1),
                        perf_mode=DR, skip_group_check=True)
    for ps, dst_n, biased in ((pro[0], 0, True), (pro[1], None, False),
                              (pro[2], 1, True)):
        if biased:
            nc.vector.tensor_scalar(
                Q8[:, 0, dst_n * 512:(dst_n + 1) * 512], ps,
                bqc_sb[:, 0:1], 1.0 / 64.0,
                mybir.AluOpType.add, mybir.AluOpType.mult)
        else:
            nc.vector.tensor_scalar_mul(K8[:, 0, 0:512], ps, 1.0 / 64.0)

    # weighted fillers (weights ~ PE kilocycles), ordered by need: K(0,n)
    # feeds pass-1 iteration 4n, V(m) feeds pass-1 iteration m+lag, later
    # e-tile/chunk projections feed passes 3..8
    QW, KW, VW, OW = 2.6, 2.0, 1.8, 2.1
    F = []
    F.append((KW, ("K", 0, 1), (proj_qk, (*KG, 0, 1))))
    F.extend((VW, ("V", m), (proj_v, (m,))) for m in (0, 1))
    F.append((KW, ("K", 0, 2), (proj_qk, (*KG, 0, 2))))
    F.extend((VW, ("V", m), (proj_v, (m,))) for m in (2, 3))
    F.append((KW, ("K", 0, 3), (proj_qk, (*KG, 0, 3))))
    F.extend((VW, ("V", m), (proj_v, (m,))) for m in range(4, 12))
    F.append((QW, ("Q", 0, 2), (proj_qk, (*QG, 0, 2))))
    F.append((QW, ("Q", 0, 3), (proj_qk, (*QG, 0, 3))))
    F.extend((VW, ("V", m), (proj_v, (m,))) for m in range(12, 16))
    F.append((KW, ("K", 1, 0), (proj_qk, (*KG, 1, 0))))
    F.append((KW, ("K", 1, 1), (proj_qk, (*KG, 1, 1))))
    F.append((QW, ("Q", 1, 0), (proj_qk, (*QG, 1, 0))))
    F.append((QW, ("Q", 1, 1), (proj_qk, (*QG, 1, 1))))
    F.append((KW, ("K", 1, 2), (proj_qk, (*KG, 1, 2))))
    F.append((KW, ("K", 1, 3), (proj_qk, (*KG, 1, 3))))
    F.append((QW, ("Q", 1, 2), (proj_qk, (*QG, 1, 2))))
    F.append((QW, ("Q", 1, 3), (proj_qk, (*QG, 1, 3))))
    fillers = [(w, k, None, (lambda f=f, a=a: f(*a))) for w, k, (f, a) in F]

    issued = {("K", 0, 0), ("Q", 0, 0), ("Q", 0, 1)}  # prologue
    normed = [0]

    def run_filler():
        w, k, gate, fn = fillers.pop(0)
        issued.add(k)
        fn()
        return w

    def require(key):
        # a pass may never read a projection whose filler has not been
        # ISSUED yet -- semaphores cannot order around program order
        while key not in issued:
            run_filler()

    # pass order: e-tile 0 over both chunks/heads, then e-tile 1; chunk-0
    # O-projections become available after pass 6
    bud = cfg["bud"]
    pv_state = attn_head(0, 0, 0, fillers, *bud[0], carry=cfg["carry1"])
    pv_state = attn_head(0, 0, 1, fillers, *bud[1], prev=pv_state,
                         lag=cfg["lag2"])
    pv_state = attn_head(1, 0, 0, fillers, *bud[2], prev=pv_state)
    pv_state = attn_head(1, 0, 1, fillers, *bud[3], prev=pv_state)
    pv_state = attn_head(0, 1, 0, fillers, *bud[4], prev=pv_state)
    pv_state = attn_head(0, 1, 1, fillers, *bud[5], prev=pv_state)
    fillers.extend(
        (OW, ("O", 0, t), (lambda: normed[0] >= 6),
         (lambda t=t: o_tile(0, t, False)))
        for t in range(NCHUNK // P))
    pv_state = attn_head(1, 1, 0, fillers, *bud[6], prev=pv_state)
    pv_state = attn_head(1, 1, 1, fillers, *bud[7], prev=pv_state,
                         last=True, lag=cfg.get("lag8", 6))
    nrm = pv_state[0]
    while fillers:
        run_filler()
    # final chunk: each norm half immediately unblocks four O-tiles
    nrm.half(0)
    for t in range(4):
        o_tile(1, t, True)
    nrm.half(1)
    for t in range(4, NCHUNK // P):
        o_tile(1, t, True)


def _split_multi_waits(nc):
    """This walrus build allows exactly one sync-wait per instruction
    (the ISA EVENTS field has a single slot).  Hoist extra waits into
    same-engine NoOps placed immediately before the instruction."""
    n = 0
    for f in nc.m.functions:
        for blk in f.blocks:
            out = []
            for inst in blk.instructions:
                si = getattr(inst, "sync_info", None)
                if si is not None and si.on_wait and len(si.on_wait) > 1:
                    waits = list(si.on_wait)
                    for w in waits[:-1]:
                        n += 1
                        out.append(mybir.InstNoOp(
                            name=f"I-wsplit-{n}",
                            engine=inst.engine,
                            ins=[], outs=[],
                            sync_info=mybir.SyncInfo(on_wait=[w], on_update=[]),
                        ))
                    si.on_wait = waits[-1:]
                out.append(inst)
            blk.instructions = out
    return n


_NC_CACHE = None
_SPLIT_WAITS = True  # walrus needs single-wait instructions; CoreSim doesn't


def _build_nc():
    global _NC_CACHE
    if _NC_CACHE is not None:
        return _NC_CACHE
    # disable_frame_to_traceback keeps source paths out of the BIR so the
    # neuron compile cache hits regardless of which directory kernel.py
    # runs from
    nc = bass.Bass("TRN2", target_bir_lowering=False, debug=False,
                   disable_frame_to_traceback=True)
    ins = {
        "xh": nc.dram_tensor("xh", [P, KD, NTOK], F8, kind="ExternalInput").ap(),
        "xl": nc.dram_tensor("xl", [P, KD, NTOK], F8, kind="ExternalInput").ap(),
        "wqh": nc.dram_tensor("wqh", [P, KD, 2 * P], F8, kind="ExternalInput").ap(),
        "wkh": nc.dram_tensor("wkh", [P, KD, 2 * P], F8, kind="ExternalInput").ap(),
        "wvh": nc.dram_tensor("wvh", [P, KD, 2 * P], F8, kind="ExternalInput").ap(),
        "wvl": nc.dram_tensor("wvl", [P, KD, 2 * P], F8, kind="ExternalInput").ap(),
        "wo": nc.dram_tensor("wo", [P, NE, ED], BF16, kind="ExternalInput").ap(),
        "bq": nc.dram_tensor("bq", [P, NE], F32, kind="ExternalInput").ap(),
        "bv": nc.dram_tensor("bv", [1, 2 * P], BF16, kind="ExternalInput").ap(),
    }
    outs = {
        "out": nc.dram_tensor("out", [NTOK, ED], BF16, kind="ExternalOutput").ap(),
    }
    with tile.TileContext(nc) as tc, ExitStack() as ctx:
        _mha_body(ctx, tc, outs, ins)
    if _SPLIT_WAITS:
        _split_multi_waits(nc)
    # scrub source paths / caller frames from the BIR so it is byte-identical
    # regardless of where kernel.py lives -> neuron compile cache always hits
    for f in nc.m.functions:
        for al in f.allocations:
            mls = getattr(al, "memorylocations", None)
            if mls:
                for ml in mls:
                    if getattr(ml, "ant_debug", None) is not None:
                        ml.ant_debug = None
        for blk in f.blocks:
            for inst in blk.instructions:
                if getattr(inst, "debug", None) is not None:
                    inst.debug = None
    _NC_CACHE = nc
    return nc


def _split_fp8(a):
    """f32 array -> (hi, lo) e4m3 pair with hi + lo ~= a."""
    hi = a.astype(NP8)
    lo = (a - hi.astype(np.float32)).astype(NP8)
    return hi, lo


def _prep_weight(w_slice):
    """w[e0:e0+256, :] f32 -> two SBUF layouts [128, 8, 256] fp8 (hi, lo)."""
    wt = np.ascontiguousarray(w_slice.T) * WS          # [1024, 256]
    hi, lo = _split_fp8(wt)
    def lay(a):
        return np.ascontiguousarray(
            a.reshape(KD, P, 2 * P).transpose(1, 0, 2))
    return lay(hi), lay(lo)


def make_in_maps(x, wq, bq, wk, bk, wv, bv, wo, bo):
    x = np.asarray(x, np.float32).reshape(B, NTOK, ED)
    in_maps = []
    x_by_batch = []
    for b in range(B):
        xt = np.ascontiguousarray(x[b].T) * XS  # [1024, 2048]
        hi, lo = _split_fp8(xt)
        x_by_batch.append(tuple(
            np.ascontiguousarray(a.reshape(KD, P, NTOK).transpose(1, 0, 2))
            for a in (hi, lo)))
    bq1024 = np.asarray(bq, np.float32) * PSC
    bv1024 = (np.asarray(bv, np.float32) * PSC).astype(NPBF16)
    for c in range(8):
        b = c // 4
        e0 = (c % 4) * 256
        wqh, _ = _prep_weight(np.asarray(wq, np.float32)[e0:e0 + 256])
        wkh, _ = _prep_weight(np.asarray(wk, np.float32)[e0:e0 + 256])
        wvh, wvl = _prep_weight(np.asarray(wv, np.float32)[e0:e0 + 256])
        wo_sl = np.ascontiguousarray(np.asarray(wo, np.float32)[:, e0:e0 + 256].T)
        in_maps.append({
            "xh": x_by_batch[b][0], "xl": x_by_batch[b][1],
            "wqh": wqh,
            "wkh": wkh,
            "wvh": wvh, "wvl": wvl,
            "wo": np.ascontiguousarray(
                wo_sl.reshape(NE, P, ED).transpose(1, 0, 2)).astype(NPBF16),
            "bq": np.ascontiguousarray(
                bq1024[e0:e0 + 256].reshape(NE, P).T),
            "bv": bv1024[e0:e0 + 256].reshape(1, 2 * P),
        })
    return in_maps


_FN_CACHE = None


def _build_fn(nc, n_cores=8):
    """Multi-core PJRT executor (mirrors bass2jax.run_bass_via_pjrt's
    shard_map path, minus buffer donation so the jitted callable can be
    cached and reused across kernel() calls)."""
    import jax
    from jax.sharding import Mesh, PartitionSpec
    from jax.experimental.shard_map import shard_map
    import concourse.bass2jax as b2j
    from concourse import mybir

    b2j.install_neuronx_cc_hook()
    pname = nc.partition_id_tensor.name if nc.partition_id_tensor else None
    in_names, out_names, out_avals = [], [], []
    for alloc in nc.m.functions[0].allocations:
        if not isinstance(alloc, mybir.MemoryLocationSet):
            continue
        name = alloc.memorylocations[0].name
        if alloc.kind == "ExternalInput":
            if name != pname:
                in_names.append(name)
        elif alloc.kind == "ExternalOutput":
            out_names.append(name)
            out_avals.append(jax.core.ShapedArray(
                tuple(alloc.tensor_shape), mybir.dt.np(alloc.dtype)))
    n_params = len(in_names)
    all_in = list(in_names) + list(out_names)
    if pname is not None:
        all_in.append(pname)

    def _body(*args):
        ops = list(args)
        if pname is not None:
            ops.append(b2j.partition_id_tensor())
        return tuple(b2j._bass_exec_p.bind(
            *ops,
            out_avals=tuple(out_avals), in_names=tuple(all_in),
            out_names=tuple(out_names), lowering_input_output_aliases=(),
            sim_require_finite=True, sim_require_nnan=True, nc=nc))

    devices = jax.devices()[:n_cores]
    mesh = Mesh(np.asarray(devices), ("core",))
    specs = (PartitionSpec("core"),) * (n_params + len(out_names))
    fn = jax.jit(shard_map(_body, mesh=mesh, in_specs=specs,
                           out_specs=(PartitionSpec("core"),) * len(out_names),
                           check_rep=False))
    zeros = [np.zeros((n_cores * a.shape[0], *a.shape[1:]), a.dtype)
             for a in out_avals]
    return fn, in_names, zeros


def kernel(x, wq, bq, wk, bk, wv, bv, wo, bo, **_ignored):
    global _FN_CACHE
    nc = _build_nc()
    in_maps = make_in_maps(x, wq, bq, wk, bk, wv, bv, wo, bo)
    if _FN_CACHE is None:
        _FN_CACHE = _build_fn(nc)
    fn, in_names, zeros = _FN_CACHE
    concat_in = [np.concatenate([in_maps[c][n] for c in range(8)], axis=0)
                 for n in in_names]
    outs = fn(*concat_in, *zeros)
    o = np.asarray(outs[0]).astype(np.float32).reshape(8, NTOK, ED)
    bo = np.asarray(bo, np.float32)
    out = np.empty((B, NTOK, ED), np.float32)
    for b in range(B):
        out[b] = o[4 * b:4 * b + 4].sum(axis=0) + bo
    return out


# revision 37
# speedup vs baseline: 1.0049x; 1.0049x over previous
"""Multi-head attention forward on 8 TRN2 NeuronCores.

Problem: x[2, 2048, 1024], 16 heads x 64 dims, nn.Linear-style Q/K/V/O
projections.

Sharding: core c owns batch b = c // 4 and heads [4*(c%4), 4*(c%4)+4).
Each core computes Q/K/V projections for its 4 heads over its batch's
2048 tokens, attention, and a partial O-projection restricted to its
heads' input dims.  The host sums the 4 partials per batch and adds bo.

Precision scheme (validated on HW, end-to-end rel err ~9e-3 vs 2e-2):
  - x ships as e4m3 hi/lo pairs scaled 16x; Q/K/V weights as e4m3
    scaled 64x (the uniform(+-1/32) weights would otherwise sit below
    e4m3's 2^-6 min-normal).  Q/K projections run 2-term compensated
    fp8 DoubleRow matmuls (w_hi*(x_hi + x_lo)); V runs 3-term
    (+ w_lo*x_hi) since it is not re-quantized.  All cheaper than bf16
    (0.5 cycles/row in DoubleRow mode).
  - Q and K are re-quantized to e4m3 at 16x scale on copyback
    (psum * 1/64, Q with fused per-partition bias add); the 2048-wide
    scores matmuls run fp8 DoubleRow with the head dim split as one
    real 64-row plane plus a zeroed "ghost" plane, halving PE time vs
    bf16.  The exp scale absorbs the 256x.
  - K projection bias is dropped entirely: softmax over keys is
    invariant to the per-query q.bk term, and the bq.k term is kept via
    Q's bias.  attn weights (exp) and V stay bf16 -- fp8 there fails
    the error budget.
  - O projection bf16; per-core partials stored bf16, summed f32 host.

On-chip layout / schedule (per core):
  - Q8/K8 "transposed": [hd on partitions, tokens free] per e-tile;
    K8 carries a zeroed 128-col block at the end (stationary ghost
    plane), Q8 a zeroed 256-col pad (moving-operand lookahead).
  - V produced directly as [token, dim] tiles (stationary x trick), no
    PE transposes; a ones-column per head makes the PV matmul emit the
    softmax denominators as row 64 of the ctx PSUM tile.
  - 8 single-head passes (16 m-iterations each); PSUM: two score banks
    (s0/s1), one ctx accumulator (cx), two 1-bank projection slots
    (p0/p1).  Projection/O-tile "fillers" are paced into the PE queue by
    a budget, with issue-order dependencies enforced by require().
    Each pass defers its last PVs and its normalization into the next
    pass so no pass ends in a drain burst that starves the ACT engine
    (exp is the second-busiest engine and must run back-to-back).
  - denominator reciprocals (bf16) are broadcast across 64 partitions
    with a K=1 ones matmul, ctx normalized into CT, then O-projection.
"""

import json
import os
from contextlib import ExitStack

import ml_dtypes
import numpy as np

import concourse.bass as bass
import concourse.tile as tile
from concourse import mybir

BF16 = mybir.dt.bfloat16
F8 = mybir.dt.float8e4
F32 = mybir.dt.float32
AF = mybir.ActivationFunctionType
DR = mybir.MatmulPerfMode.DoubleRow
NPBF16 = ml_dtypes.bfloat16
NP8 = ml_dtypes.float8_e4m3

P = 128
B = 2
NTOK = 2048          # tokens per core (one batch)
ED = 1024
KD = ED // P         # 8 contraction k-tiles for projections
NE = 2               # e-tiles per core (4 heads * 64 = 256 dims)
NH_CORE = 4          # heads per core
HD = 64
MT = NTOK // P       # 16 key/value m-tiles
NCHUNK = 1024        # query-token chunk for the attention inner loop
VROW = NH_CORE * 65  # V tile row: 4x (64 dims + ones column)
XS, WS = 16.0, 64.0  # fp8 pre-scales on x and w
PSC = XS * WS        # projection psum scale
EXPSC = 0.125 / (XS * XS)  # exp scale: 1/sqrt(64) / (16*16)


def _mha_body(ctx: ExitStack, tc: tile.TileContext, outs: dict, ins: dict):
    nc = tc.nc
    xh, xl = ins["xh"], ins["xl"]                   # [128, 8, 2048] fp8
    wq, wk = ins["wqh"], ins["wkh"]                 # [128, 8, 256] fp8
    wv = (ins["wvh"], ins["wvl"])
    wo = ins["wo"]          # [128, 2, 1024] bf16   [p, k, d] = wo[d, e0+128k+p]
    bq, bv = ins["bq"], ins["bv"]                   # [1, 256] bf16, x1024
    out = outs["out"]       # [2048, 1024] f32

    const = ctx.enter_context(tc.tile_pool(name="const", bufs=1))
    sb_big = ctx.enter_context(tc.tile_pool(name="sb_big", bufs=1))
    sb_ex = ctx.enter_context(tc.tile_pool(name="sb_ex", bufs=14))
    sb_sm = ctx.enter_context(tc.tile_pool(name="sb_sm", bufs=4))
    sb_out = ctx.enter_context(tc.tile_pool(name="sb_out", bufs=4))
    psum = ctx.enter_context(tc.tile_pool(name="psum", bufs=1, space="PSUM"))

    # ---- SBUF homes ----
    xh_sb = sb_big.tile([P, KD, NTOK], F8)
    xl_sb = sb_big.tile([P, KD, NTOK], F8)
    wqh_sb = const.tile([P, KD, 2 * P], F8)
    wkh_sb = const.tile([P, KD, 2 * P], F8)
    wvh_sb = const.tile([P, KD, 2 * P], F8)
    wvl_sb = const.tile([P, KD, 2 * P], F8)
    wo_sb = const.tile([P, NE, ED], BF16)
    bqc_sb = const.tile([P, NE], F32)
    bv_sb = const.tile([1, 2 * P], BF16)

    ones_row = const.tile([1, 512], BF16)
    Q8 = sb_big.tile([P, NE, NTOK + 256], F8)   # pad: moving ghost lookahead
    K8 = sb_big.tile([P, NE, NTOK + P], F8)     # pad: stationary ghost zeros
    V = sb_big.tile([P, MT, VROW], BF16)
    CT = sb_big.tile([P, NE, NTOK], BF16)  # normalized ctxT[e, n]
    nc.vector.memset(ones_row, 1.0)
    nc.vector.memset(V[:, :, 64::65], 1.0)
    nc.vector.memset(Q8[:, :, NTOK:], 0.0)
    nc.vector.memset(K8[:, :, NTOK:], 0.0)

    # ---- input DMA on two parallel HWDGE rings (SP + ACT), batched and
    # ordered so the critical prologue path (wk, wq, x tokens 0:1024) lands
    # first on both rings.
    nc.sync.dma_start(xh_sb[:, :, 0:512], xh[:, :, 0:512])
    nc.sync.dma_start(xl_sb[:, :, 0:512], xl[:, :, 0:512])
    nc.sync.dma_start(bqc_sb, bq)
    nc.sync.dma_start(bv_sb, bv)
    nc.sync.dma_start(wvh_sb, wv[0])
    nc.sync.dma_start(wvl_sb, wv[1])
    nc.sync.dma_start(xh_sb[:, :, 1024:1536], xh[:, :, 1024:1536])
    nc.sync.dma_start(xl_sb[:, :, 1024:1536], xl[:, :, 1024:1536])
    nc.scalar.dma_start(wqh_sb, wq)
    nc.scalar.dma_start(wkh_sb, wk)
    nc.scalar.dma_start(xh_sb[:, :, 512:1024], xh[:, :, 512:1024])
    nc.scalar.dma_start(xl_sb[:, :, 512:1024], xl[:, :, 512:1024])
    nc.scalar.dma_start(wo_sb, wo)
    nc.scalar.dma_start(xh_sb[:, :, 1536:2048], xh[:, :, 1536:2048])
    nc.scalar.dma_start(xl_sb[:, :, 1536:2048], xl[:, :, 1536:2048])


    pj = [0]
    TERMS = ((0, 0), (0, 1), (1, 0))  # (w, x) hi/lo compensation pairs
    cfg = json.loads(os.environ.get("KTUNE", "null")) or {
        "bud": [[1.8, 0.9], [1.0, 0.9], [0.9, 0.8], [0.9, 0.8],
                [0.9, 0.8], [0.8, 0.8], [1.0, 1.1], [1.0, 1.1]],
        "carry1": 6, "lag2": 7, "lag8": 6, "wrm": 8}

    def q_ghost(t, i, woff):
        """Q8 window as [64, 2, 256] with plane 1 -> the zero pad."""
        s = Q8[64 * i:64 * i + 64, t, woff:woff + 512]
        return bass.AP(tensor=s.tensor, offset=s.offset,
                       ap=[s.ap[0], [NTOK - woff, 2], [1, 256]])

    def k_ghost(t, i, m):
        """K8 m-tile as [64, 2, 128] with plane 1 -> the zero block."""
        s = K8[64 * i:64 * i + 64, t, m * P:(m + 1) * P]
        return bass.AP(tensor=s.tensor, offset=s.offset,
                       ap=[s.ap[0], [NTOK - m * P, 2], [1, P]])

    def proj_qk(w_sb, b_sb, dst, t, n):
        """Q/K projection tile: dst[:, t, n*512:(n+1)*512] fp8 = psum/64.

        2-term compensation: w_hi @ (x_hi + x_lo); the dropped w_lo term
        is below the e4m3 re-quantization noise of the QK matmul."""
        ps = psum.tile([P, 512], F32, tag=f"p{pj[0] % 2}", name="ps_proj")
        pj[0] += 1
        x2 = (xh_sb, xl_sb)
        for b in range(2):
            for w in range(2):
                base = n * 512 + w * 256
                for j in range(KD // 2):
                    nc.tensor.matmul(
                        ps[:, w * 256:(w + 1) * 256],
                        w_sb[:, 2 * j:2 * j + 2, t * P:(t + 1) * P],
                        x2[b][:, 2 * j:2 * j + 2, base:base + 256],
                        start=(b == 0 and w == 0 and j == 0),
                        stop=(b == 1 and w == 1 and j == KD // 2 - 1),
                        perf_mode=DR, skip_group_check=True)
        if b_sb is not None:
            # fused (psum + 1024*bq) / 64 on the copyback
            nc.vector.tensor_scalar(
                dst[:, t, n * 512:(n + 1) * 512], ps,
                b_sb[:, t:t + 1], 1.0 / 64.0,
                mybir.AluOpType.add, mybir.AluOpType.mult)
        else:
            nc.vector.tensor_scalar_mul(
                dst[:, t, n * 512:(n + 1) * 512], ps, 1.0 / 64.0)

    def proj_v(m):
        """V m-tile direct as [token, dim]: V[:, m, :] bf16 = psum/1024."""
        pv = psum.tile([P, 2 * P], F32, tag=f"p{pj[0] % 2}", name="ps_v")
        pj[0] += 1
        x2 = (xh_sb, xl_sb)
        for ti, (a, b) in enumerate(TERMS):
            for j in range(KD // 2):
                nc.tensor.matmul(
                    pv,
                    x2[b][:, 2 * j:2 * j + 2, m * P:(m + 1) * P],
                    (wvh_sb, wvl_sb)[a][:, 2 * j:2 * j + 2, :],
                    start=(ti == 0 and j == 0), stop=False, perf_mode=DR)
        nc.tensor.matmul(pv, ones_row[:, 0:P], bv_sb,
                         start=False, stop=True)
        s = V[:, m, 0:VROW]
        dst = bass.AP(tensor=s.tensor, offset=s.offset,
                      ap=[s.ap[0], [65, NH_CORE], [1, HD]])
        nc.vector.tensor_scalar_mul(dst, pv, 1.0 / PSC)

    def o_tile(c, t, tail):
        r = c * NCHUNK + t * P
        ob = sb_out.tile([P, ED], BF16)
        slots = ("p0", "p1", "s0", "s1") if tail else ("p0", "p1")
        for u in range(2):
            po = psum.tile([P, 512], F32, tag=slots[pj[0] % len(slots)],
                           name="ps_o2")
            pj[0] += 1
            for k in range(NE):
                nc.tensor.matmul(
                    po,
                    CT[:, k, r:r + P],
                    wo_sb[:, k, u * 512:(u + 1) * 512],
                    start=(k == 0), stop=(k == NE - 1))
            if u == 1 and tail:
                # ACT is idle once the last exp has issued
                nc.scalar.copy(ob[:, u * 512:(u + 1) * 512], po)
            else:
                nc.vector.tensor_copy(ob[:, u * 512:(u + 1) * 512], po)
        eng = nc.sync if t % 2 == 0 else nc.scalar
        eng.dma_start(out[r:r + P, :], ob)

    KG = (wkh_sb, None, K8)
    QG = (wqh_sb, bqc_sb, Q8)

    def attn_head(c, et, i, fillers, budget_early, budget, lag=6,
                  prev=None, last=False, carry=4):
        """One head-chunk pass: head 2*et+i over query chunk c.

        Returns (norm, leftover): `leftover` is this pass's last CARRY pv
        closures, `norm` issues its normalization.  The NEXT pass runs the
        leftover pvs in its first two iterations and the norm right after,
        so no pass ends with a PE drain burst that starves ACT, and the
        single cx PSUM buffer is handed over exactly between the norm
        reads and the next first PV (which waits at least `lag` iters)."""
        g = c * NCHUNK
        h = 2 * et + i
        cx = psum.tile([65, NCHUNK], F32, tag="cx", bufs=1, name="ps_cx")
        exs = {}
        pvd = [0]
        own_last = MT if last else MT - carry

        def pv(m):
            ex = exs.pop(m)
            for u in range(2):
                nc.tensor.matmul(
                    cx[:, u * 512:(u + 1) * 512],
                    V[:, m, h * 65:h * 65 + 65],
                    ex[:, u * 512:(u + 1) * 512],
                    start=(m == 0), stop=(m == MT - 1))

        debt = [0.0]
        require(("Q", et, 2 * c))
        require(("Q", et, 2 * c + 1))
        for m in range(MT):
            require(("K", et, m // 4))
            sc = psum.tile([P, NCHUNK], F32, tag=f"s{m % 2}", name="ps_sc")
            for j in range(4):
                nc.tensor.matmul(
                    sc[:, j * 256:(j + 1) * 256],
                    k_ghost(et, i, m),
                    q_ghost(et, i, g + 256 * j),
                    start=(j % 2 == 0), stop=(j % 2 == 1), perf_mode=DR,
                    skip_group_check=True)
            ex = sb_ex.tile([P, NCHUNK], BF16, tag="ex", name="ex")
            nc.scalar.activation(ex, sc, AF.Exp, scale=EXPSC)
            exs[m] = ex
            if prev is not None:
                if prev[1]:
                    # spread carried PVs: one per iteration when they fit in
                    # the first six iterations (each may force-drain its V
                    # projection -- two per iteration starves ACT)
                    n = 1 if len(prev[1]) <= 6 - m else 2
                    for _ in range(n):
                        if prev[1]:
                            prev[1].pop(0)()
                elif prev[0] is not None:
                    prev[0]()
                    prev = (None, ())
            # own PVs: lag behind the exp stream, catching up to two per
            # iteration late in the pass, leaving the last CARRY deferred
            target = min(own_last, m + 1,
                         max(0, m - lag + 1,
                             2 * m - 2 * (MT - 2) + own_last))
            while pvd[0] < target:
                require(("V", pvd[0]))
                pv(pvd[0])
                pvd[0] += 1
            debt[0] += budget_early if m < lag else budget
            # keep the DVE queue clear near the pass end so the
            # normalization chain is not stuck behind filler copybacks
            if (fillers and m < MT - 3 and debt[0] >= fillers[0][0]
                    and (fillers[0][2] is None or fillers[0][2]())):
                debt[0] -= run_filler()
        while last and pvd[0] < MT:
            require(("V", pvd[0]))
            pv(pvd[0])
            pvd[0] += 1
        leftover = []
        for m in range(pvd[0], MT):
            def run_pv(m=m):
                require(("V", m))
                pv(m)
            leftover.append(run_pv)

        def norm_half(v):
            # normalization, pipelined in column halves; the reciprocal row
            # is broadcast across 64 partitions with a K=1 ones matmul (PE)
            pp = i * HD
            vs = slice(v * 512, (v + 1) * 512)
            rcp = sb_sm.tile([1, 512], BF16, tag="rcp", name="rcp")
            with nc.allow_low_precision(reason="bf16 denominators: "
                                        "0.4% rms, inside the budget"):
                nc.vector.reciprocal(rcp, cx[64:65, vs])
            bcp = psum.tile([HD, 512], F32, tag=f"p{pj[0] % 2}",
                            name="ps_bc")
            pj[0] += 1
            nc.tensor.matmul(bcp, ones_row[:, 0:HD], rcp,
                             start=True, stop=True)
            bc = sb_sm.tile([HD, 512], BF16, name="bc")
            nc.vector.tensor_copy(bc, bcp)
            nc.vector.tensor_mul(
                CT[pp:pp + HD, et, g + v * 512:g + v * 512 + 512],
                cx[0:HD, vs], bc)

        def norm():
            norm_half(0)
            norm_half(1)
            normed[0] += 1
        norm.half = norm_half
        return norm, leftover

    # PE warmup: dependency-free dummy matmuls burn through the p-state
    # ramp while the first input DMAs land, so the prologue projections
    # run at full clock
    wrm = psum.tile([P, 512], F32, tag="p0", name="wrm")
    for _ in range(cfg.get("wrm", 8)):
        nc.tensor.matmul(wrm, ones_row[:, 0:P], ones_row,
                         start=True, stop=True)
    pj[0] += 1  # leave p1 for the first projection

    # prologue: the minimum the first pass (head 0, e-tile 0, chunk 0)
    # needs: K(0,0), Q(0,0), Q(0,1).  Emitted with all x_hi phases first so
    # PE is never parked on the later x_lo DMAs.
    x2 = (xh_sb, xl_sb)
    pro = []
    for tag, w_sb, n in (("p0", wqh_sb, 0), ("p1", wkh_sb, 0),
                         ("s0", wqh_sb, 1)):
        pro.append(psum.tile([P, 512], F32, tag=tag, name="ps_proj"))
    for b in range(2):
        for (ps, w_sb, n) in zip(pro, (wqh_sb, wkh_sb, wqh_sb), (0, 0, 1)):
            for w in range(2):
                base = n * 512 + w * 256
                for j in range(KD // 2):
                    nc.tensor.matmul(
                        ps[:, w * 256:(w + 1) * 256],
                        w_sb[:, 2 * j:2 * j + 2, 0:P],
                        x2[b][:, 2 * j:2 * j + 2, base:base + 256],
                        start=(b == 0 and w == 0 and j == 0),
                        stop=(b == 1 and w == 1 and j == KD // 2 - 1),
                        perf_mode=DR, skip_group_check=True)
    for ps, dst_n, biased in ((pro[0], 0, True), (pro[1], None, False),
                              (pro[2], 1, True)):
        if biased:
            nc.vector.tensor_scalar(
                Q8[:, 0, dst_n * 512:(dst_n + 1) * 512], ps,
                bqc_sb[:, 0:1], 1.0 / 64.0,
                mybir.AluOpType.add, mybir.AluOpType.mult)
        else:
            nc.vector.tensor_scalar_mul(K8[:, 0, 0:512], ps, 1.0 / 64.0)

    # weighted fillers (weights ~ PE kilocycles), ordered by need: K(0,n)
    # feeds pass-1 iteration 4n, V(m) feeds pass-1 iteration m+lag, later
    # e-tile/chunk projections feed passes 3..8
    QW, KW, VW, OW = 2.6, 2.0, 1.8, 2.1
    F = []
    F.append((KW, ("K", 0, 1), (proj_qk, (*KG, 0, 1))))
    F.extend((VW, ("V", m), (proj_v, (m,))) for m in (0, 1))
    F.append((KW, ("K", 0, 2), (proj_qk, (*KG, 0, 2))))
    F.extend((VW, ("V", m), (proj_v, (m,))) for m in (2, 3))
    F.append((KW, ("K", 0, 3), (proj_qk, (*KG, 0, 3))))
    F.extend((VW, ("V", m), (proj_v, (m,))) for m in range(4, 12))
    F.append((QW, ("Q", 0, 2), (proj_qk, (*QG, 0, 2))))
    F.append((QW, ("Q", 0, 3), (proj_qk, (*QG, 0, 3))))
    F.extend((VW, ("V", m), (proj_v, (m,))) for m in range(12, 16))
    F.append((KW, ("K", 1, 0), (proj_qk, (*KG, 1, 0))))
    F.append((KW, ("K", 1, 1), (proj_qk, (*KG, 1, 1))))
    F.append((QW, ("Q", 1, 0), (proj_qk, (*QG, 1, 0))))
    F.append((QW, ("Q", 1, 1), (proj_qk, (*QG, 1, 1))))
    F.append((KW, ("K", 1, 2), (proj_qk, (*KG, 1, 2))))
    F.append((KW, ("K", 1, 3), (proj_qk, (*KG, 1, 3))))
    F.append((QW, ("Q", 1, 2), (proj_qk, (*QG, 1, 2))))
    F.append((QW, ("Q", 1, 3), (proj_qk, (*QG, 1, 3))))
    fillers = [(w, k, None, (lambda f=f, a=a: f(*a))) for w, k, (f, a) in F]

    issued = {("K", 0, 0), ("Q", 0, 0), ("Q", 0, 1)}  # prologue
    normed = [0]

    def run_filler():
        w, k, gate, fn = fillers.pop(0)
        issued.add(k)
        fn()
        return w

    def require(key):
        # a pass may never read a projection whose filler has not been
        # ISSUED yet -- semaphores cannot order around program order
        while key not in issued:
            run_filler()

    # pass order: e-tile 0 over both chunks/heads, then e-tile 1; chunk-0
    # O-projections become available after pass 6
    bud = cfg["bud"]
    pv_state = attn_head(0, 0, 0, fillers, *bud[0], carry=cfg["carry1"])
    pv_state = attn_head(0, 0, 1, fillers, *bud[1], prev=pv_state,
                         lag=cfg["lag2"])
    pv_state = attn_head(1, 0, 0, fillers, *bud[2], prev=pv_state)
    pv_state = attn_head(1, 0, 1, fillers, *bud[3], prev=pv_state)
    pv_state = attn_head(0, 1, 0, fillers, *bud[4], prev=pv_state)
    pv_state = attn_head(0, 1, 1, fillers, *bud[5], prev=pv_state)
    fillers.extend(
        (OW, ("O", 0, t), (lambda: normed[0] >= 6),
         (lambda t=t: o_tile(0, t, False)))
        for t in range(NCHUNK // P))
    pv_state = attn_head(1, 1, 0, fillers, *bud[6], prev=pv_state)
    pv_state = attn_head(1, 1, 1, fillers, *bud[7], prev=pv_state,
                         last=True, lag=cfg.get("lag8", 6))
    nrm = pv_state[0]
    while fillers:
        run_filler()
    # final chunk: each norm half immediately unblocks four O-tiles
    nrm.half(0)
    for t in range(4):
        o_tile(1, t, True)
    nrm.half(1)
    for t in range(4, NCHUNK // P):
        o_tile(1, t, True)


def _split_multi_waits(nc):
    """This walrus build allows exactly one sync-wait per instruction
    (the ISA EVENTS field has a single slot).  Hoist extra waits into
    same-engine NoOps placed immediately before the instruction."""
    n = 0
    for f in nc.m.functions:
        for blk in f.blocks:
            out = []
            for inst in blk.instructions:
                si = getattr(inst, "sync_info", None)
                if si is not None and si.on_wait and len(si.on_wait) > 1:
                    waits = list(si.on_wait)
                    for w in waits[:-1]:
                        n += 1
                        out.append(mybir.InstNoOp(
                            name=f"I-wsplit-{n}",
                            engine=inst.engine,
                            ins=[], outs=[],
                            sync_info=mybir.SyncInfo(on_wait=[w], on_update=[]),
                        ))
                    si.on_wait = waits[-1:]
                out.append(inst)
            blk.instructions = out
    return n


_NC_CACHE = None
_SPLIT_WAITS = True  # walrus needs single-wait instructions; CoreSim doesn't


def _build_nc():
    global _NC_CACHE
    if _NC_CACHE is not None:
        return _NC_CACHE
    # disable_frame_to_traceback keeps source paths out of the BIR so the
    # neuron compile cache hits regardless of which directory kernel.py
    # runs from
    nc = bass.Bass("TRN2", target_bir_lowering=False, debug=False,
                   disable_frame_to_traceback=True)
    ins = {
        "xh": nc.dram_tensor("xh", [P, KD, NTOK], F8, kind="ExternalInput").ap(),
        "xl": nc.dram_tensor("xl", [P, KD, NTOK], F8, kind="ExternalInput").ap(),
        "wqh": nc.dram_tensor("wqh", [P, KD, 2 * P], F8, kind="ExternalInput").ap(),
        "wkh": nc.dram_tensor("wkh", [P, KD, 2 * P], F8, kind="ExternalInput").ap(),
        "wvh": nc.dram_tensor("wvh", [P, KD, 2 * P], F8, kind="ExternalInput").ap(),
        "wvl": nc.dram_tensor("wvl", [P, KD, 2 * P], F8, kind="ExternalInput").ap(),
        "wo": nc.dram_tensor("wo", [P, NE, ED], BF16, kind="ExternalInput").ap(),
        "bq": nc.dram_tensor("bq", [P, NE], F32, kind="ExternalInput").ap(),
        "bv": nc.dram_tensor("bv", [1, 2 * P], BF16, kind="ExternalInput").ap(),
    }
    outs = {
        "out": nc.dram_tensor("out", [NTOK, ED], BF16, kind="ExternalOutput").ap(),
    }
    with tile.TileContext(nc) as tc, ExitStack() as ctx:
        _mha_body(ctx, tc, outs, ins)
    if _SPLIT_WAITS:
        _split_multi_waits(nc)
    # scrub source paths / caller frames from the BIR so it is byte-identical
    # regardless of where kernel.py lives -> neuron compile cache always hits
    for f in nc.m.functions:
        for al in f.allocations:
            mls = getattr(al, "memorylocations", None)
            if mls:
                for ml in mls:
                    if getattr(ml, "ant_debug", None) is not None:
                        ml.ant_debug = None
        for blk in f.blocks:
            for inst in blk.instructions:
                if getattr(inst, "debug", None) is not None:
                    inst.debug = None
    _NC_CACHE = nc
    return nc


def _split_fp8(a):
    """f32 array -> (hi, lo) e4m3 pair with hi + lo ~= a."""
    hi = a.astype(NP8)
    lo = (a - hi.astype(np.float32)).astype(NP8)
    return hi, lo


def _prep_weight(w_slice):
    """w[e0:e0+256, :] f32 -> two SBUF layouts [128, 8, 256] fp8 (hi, lo)."""
    wt = np.ascontiguousarray(w_slice.T) * WS          # [1024, 256]
    hi, lo = _split_fp8(wt)
    def lay(a):
        return np.ascontiguousarray(
            a.reshape(KD, P, 2 * P).transpose(1, 0, 2))
    return lay(hi), lay(lo)


def make_in_maps(x, wq, bq, wk, bk, wv, bv, wo, bo):
    x = np.asarray(x, np.float32).reshape(B, NTOK, ED)
    in_maps = []
    x_by_batch = []
    for b in range(B):
        xt = np.ascontiguousarray(x[b].T) * XS  # [1024, 2048]
        hi, lo = _split_fp8(xt)
        x_by_batch.append(tuple(
            np.ascontiguousarray(a.reshape(KD, P, NTOK).transpose(1, 0, 2))
            for a in (hi, lo)))
    bq1024 = np.asarray(bq, np.float32) * PSC
    bv1024 = (np.asarray(bv, np.float32) * PSC).astype(NPBF16)
    for c in range(8):
        b = c // 4
        e0 = (c % 4) * 256
        wqh, _ = _prep_weight(np.asarray(wq, np.float32)[e0:e0 + 256])
        wkh, _ = _prep_weight(np.asarray(wk, np.float32)[e0:e0 + 256])
        wvh, wvl = _prep_weight(np.asarray(wv, np.float32)[e0:e0 + 256])
        wo_sl = np.ascontiguousarray(np.asarray(wo, np.float32)[:, e0:e0 + 256].T)
        in_maps.append({
            "xh": x_by_batch[b][0], "xl": x_by_batch[b][1],
            "wqh": wqh,
            "wkh": wkh,
            "wvh": wvh, "wvl": wvl,
            "wo": np.ascontiguousarray(
                wo_sl.reshape(NE, P, ED).transpose(1, 0, 2)).astype(NPBF16),
            "bq": np.ascontiguousarray(
                bq1024[e0:e0 + 256].reshape(NE, P).T),
            "bv": bv1024[e0:e0 + 256].reshape(1, 2 * P),
        })
    return in_maps


_FN_CACHE = None


def _build_fn(nc, n_cores=8):
    """Multi-core PJRT executor (mirrors bass2jax.run_bass_via_pjrt's
    shard_map path, minus buffer donation so the jitted callable can be
    cached and reused across kernel() calls)."""
    import jax
    from jax.sharding import Mesh, PartitionSpec
    from jax.experimental.shard_map import shard_map
    import concourse.bass2jax as b2j
    from concourse import mybir

    b2j.install_neuronx_cc_hook()
    pname = nc.partition_id_tensor.name if nc.partition_id_tensor else None
    in_names, out_names, out_avals = [], [], []
    for alloc in nc.m.functions[0].allocations:
        if not isinstance(alloc, mybir.MemoryLocationSet):
            continue
        name = alloc.memorylocations[0].name
        if alloc.kind == "ExternalInput":
            if name != pname:
                in_names.append(name)
        elif alloc.kind == "ExternalOutput":
            out_names.append(name)
            out_avals.append(jax.core.ShapedArray(
                tuple(alloc.tensor_shape), mybir.dt.np(alloc.dtype)))
    n_params = len(in_names)
    all_in = list(in_names) + list(out_names)
    if pname is not None:
        all_in.append(pname)

    def _body(*args):
        ops = list(args)
        if pname is not None:
            ops.append(b2j.partition_id_tensor())
        return tuple(b2j._bass_exec_p.bind(
            *ops,
            out_avals=tuple(out_avals), in_names=tuple(all_in),
            out_names=tuple(out_names), lowering_input_output_aliases=(),
            sim_require_finite=True, sim_require_nnan=True, nc=nc))

    devices = jax.devices()[:n_cores]
    mesh = Mesh(np.asarray(devices), ("core",))
    specs = (PartitionSpec("core"),) * (n_params + len(out_names))
    fn = jax.jit(shard_map(_body, mesh=mesh, in_specs=specs,
                           out_specs=(PartitionSpec("core"),) * len(out_names),
                           check_rep=False))
    zeros = [np.zeros((n_cores * a.shape[0], *a.shape[1:]), a.dtype)
             for a in out_avals]
    return fn, in_names, zeros


def kernel(x, wq, bq, wk, bk, wv, bv, wo, bo, **_ignored):
    global _FN_CACHE
    nc = _build_nc()
    in_maps = make_in_maps(x, wq, bq, wk, bk, wv, bv, wo, bo)
    if _FN_CACHE is None:
        _FN_CACHE = _build_fn(nc)
    fn, in_names, zeros = _FN_CACHE
    concat_in = [np.concatenate([in_maps[c][n] for c in range(8)], axis=0)
                 for n in in_names]
    outs = fn(*concat_in, *zeros)
    o = np.asarray(outs[0]).astype(np.float32).reshape(8, NTOK, ED)
    bo = np.asarray(bo, np.float32)
    out = np.empty((B, NTOK, ED), np.float32)
    for b in range(B):
        out[b] = o[4 * b:4 * b + 4].sum(axis=0) + bo
    return out
